# revision 22
# baseline (speedup 1.0000x reference)
"""Autoregressive GRU on 8 TRN2 NeuronCores.

Data-parallel: batch B=512 is split as 64 rows per core; the small GRU
weights are replicated and the T=128 sequential loop runs locally per core.

Key algebra (Keras GRU, reset_after=True, gate order [z, r, h]):
  step 0:  inp = 0, h = x  ->  gx = b[0], gh = x @ U + b[1]
  step t>=1: inp == h      ->  gx + gh uses (W + U) for the z and r gates
so per step we need ONE matmul against a host-prefused weight matrix
  V  = [Wr+Ur | Wz+Uz | Uh | Wh]   (steps >= 1)   [D, 4D]
  V0 = [Ur   | Uz    | Uh | 0 ]   (step 0)       [D, 4D]
with per-gate PSUM banks in order [r, z, hh, xh], then
  r = sigmoid(rpre); z = sigmoid(zpre); hhat = tanh(xh + r*hh)
  h_new = (1-z)*hhat + z*h

Perf structure (v2 - col-tiled, fold-128 layout):
- Each M=64 matmul only fills half the 128-col PE array.  We issue the two
  256-wide halves of every gate row-block as a tile_position=(0,0)/(0,64)
  pair: the pair runs CONCURRENTLY on the two column halves of the array
  (4ns stagger), so a gate bank costs 4x~107ns instead of 4x~215ns.
- The pair's outputs land on PSUM partitions 0:64 and 64:128, i.e. every
  gate tensor is [128, 256] ("folded": partition = fold*64 + batch,
  col = feature % 256).  All elementwise work therefore runs at FD=256 on
  128 partitions - half the instruction time of the baseline's [64, 512].
- Bank order [r, z, hh, xh]: both sigmoids, u = z*h (GPSIMD) and w = 1-z
  run under the hh/xh matmul stream; the post-stream chain is only
  q = p+xh -> tanh -> m = w*hhat -> h_new = m+u -> 4 PE transposes ->
  one CAST to the fp16 stationary hT.
- Moving operands stay fp16 (exact weights, 1 cyc/row at N=256); the
  recurrent state is fp16 (~1e-2 rel overall).
- Warm-up identity transposes + two tiny regular matmuls mid-tail keep the
  PE HAM activity monitor from re-throttling the clock to 1.2 GHz.
"""

import numpy as np
import ml_dtypes

B, D, T = 512, 512, 128
NCORES = 8
BLOC = B // NCORES  # 64
P = 128
KC = D // P  # 4 K-chunks
FH = 256  # fold width (free dim of every folded [128, 256] tensor)
HH = FH // 2  # column half
GW = 4 * D  # 2048 gate columns: [r | z | hh | xh]

_FP16 = np.float16

# set by test harness to capture a profile; harmless when False
TRACE = False
TMPDIR = None
LAST = {}
# ablation flags (for debugging; all True in production)
NDUMMY = 12
WARMUP = True
DO_TRP = True


def _prepare_weights(W, U, b):
    """Host-side fusion. Gate order [r | z | hh | xh]."""
    Wz, Wr, Wh = W[:, :D], W[:, D : 2 * D], W[:, 2 * D :]
    Uz, Ur, Uh = U[:, :D], U[:, D : 2 * D], U[:, 2 * D :]
    V = np.concatenate([Wr + Ur, Wz + Uz, Uh, Wh], axis=1)  # [D, GW]
    V0 = np.concatenate([Ur, Uz, Uh, np.zeros_like(Wh)], axis=1)
    b0, b1 = b[0], b[1]
    bias = np.concatenate(
        [b0[D : 2 * D] + b1[D : 2 * D], b0[:D] + b1[:D], b1[2 * D :], b0[2 * D :]]
    )  # [GW], order [r | z | hh | xh]
    return V, V0, bias


def _dev_layout(V):
    # V_dev[p, ((k*4+g)*2+hf)*FH + c] = V[k*128+p, g*512 + hf*256 + c]
    return np.ascontiguousarray(
        V.reshape(KC, P, 4, 2, FH).transpose(1, 0, 2, 3, 4).reshape(P, KC * GW)
    )


def _fold_bias(bias):
    # folded per-gate [P, FH]: row p = fold*64+b (same for all b), col c
    out = np.zeros((4, P, FH), dtype=np.float32)
    for g in range(4):
        for hf in range(2):
            blk = bias[g * 512 + hf * 256 : g * 512 + (hf + 1) * 256]
            out[g, hf * BLOC : (hf + 1) * BLOC, :] = blk[None, :]
    return out


_CACHE = {}


def _build(has_bias: bool, T=T):
    import concourse.mybir as mybir
    import concourse.tile as tile
    from concourse import bacc
    from concourse.masks import make_identity

    f32 = mybir.dt.float32
    fp16 = mybir.dt.float16
    AF = mybir.ActivationFunctionType
    ALU = mybir.AluOpType

    nc = bacc.Bacc(
        "TRN2", target_bir_lowering=False, debug=False, num_devices=NCORES
    )
    v0_d = nc.dram_tensor("v0", [P, KC * GW], fp16, kind="ExternalInput").ap()
    v_d = nc.dram_tensor("v", [P, KC * GW], fp16, kind="ExternalInput").ap()
    h0_d = nc.dram_tensor("h0", [P, FH], fp16, kind="ExternalInput").ap()
    h0T_d = nc.dram_tensor("h0T", [P, KC * BLOC], fp16, kind="ExternalInput").ap()
    if has_bias:
        bias_d = nc.dram_tensor("bias", [4, P, FH], f32, kind="ExternalInput").ap()
    out_d = nc.dram_tensor("out", [P, T, FH], fp16, kind="ExternalOutput").ap()  # noqa: T param

    with tile.TileContext(nc) as tc:
        with (
            tc.tile_pool(name="const", bufs=1) as cpool,
            tc.tile_pool(name="state", bufs=2) as spool,
            tc.tile_pool(name="work", bufs=2) as wpool,
            tc.tile_pool(name="gates", bufs=1, space="PSUM") as gpool,
            tc.tile_pool(name="trp", bufs=1, space="PSUM") as trpool,
            tc.tile_pool(name="anc", bufs=1, space="PSUM") as ancpool,
        ):
            v0_sb = cpool.tile([P, KC * GW], fp16, tag="v0", name="v0_sb")
            v_sb = cpool.tile([P, KC * GW], fp16, tag="v", name="v_sb")
            ident = cpool.tile([P, BLOC], fp16, tag="ident", name="ident")
            nc.sync.dma_start(v0_sb[:], v0_d[:])
            make_identity(nc, ident[:BLOC, :])
            make_identity(nc, ident[BLOC:, :])

            hh_ = [spool.tile([P, HH], fp16, tag=f"h{c}", name=f"h{c}") for c in range(2)]
            hTs = [
                spool.tile([P, BLOC], fp16, tag=f"hT{k}", name=f"hT{k}")
                for k in range(KC)
            ]
            for c in range(2):
                nc.sync.dma_start(hh_[c][:], h0_d[:, c * HH : (c + 1) * HH])
            for k in range(KC):
                nc.sync.dma_start(hTs[k][:], h0T_d[:, k * BLOC : (k + 1) * BLOC])
            nc.sync.dma_start(v_sb[:], v_d[:])
            if has_bias:
                bias_sb = cpool.tile([4, P, FH], f32, tag="bias")
                nc.sync.dma_start(bias_sb[:], bias_d[:])

            # PE warm-up: dense transpose work that depends only on the
            # locally-built identity (not on any DMA) flips the HAM clock
            # gate to K=8/8 while the weight DMAs are still in flight.
            wu = trpool.tile([P, 2 * BLOC], fp16, tag="trpA", name="wu")
            for i in range(24 if WARMUP else 0):
                nc.tensor.matmul(
                    wu[:BLOC, (i % 2) * BLOC : (i % 2 + 1) * BLOC],
                    ident[:BLOC, :],
                    ident[:BLOC, :],
                    is_transpose=True,
                    start=True,
                    stop=True,
                )

            for t in range(T):
                vsb = v0_sb if t == 0 else v_sb
                last = t == T - 1
                # folded PSUM tiles: r / z banks [P, FH]; the xh bank is TWO
                # tiles (column halves) so q_A's dependency covers only the
                # 8 xh_A matmuls, not all 16 (Tile deps are tile-granular)
                gb = [
                    gpool.tile([P, FH], f32, tag=f"g{n}", name=f"g{n}")
                    for n in range(3)
                ]
                gx = [
                    gpool.tile([P, HH], f32, tag=f"gx{c}", name=f"gx{c}")
                    for c in range(2)
                ]

                KORD = (0, 2, 1, 3)  # chunk order matching the cast order

                def bank_mms(g):
                    for ki, k in enumerate(KORD):
                        for hf in range(2):
                            vbase = ((k * 4 + g) * 2 + hf) * FH
                            nc.tensor.matmul(
                                gb[g][hf * BLOC : (hf + 1) * BLOC, :],
                                hTs[k][:],
                                vsb[:, vbase : vbase + FH],
                                start=(ki == 0),
                                stop=(ki == KC - 1),
                                skip_group_check=True,
                            )
                    if has_bias:
                        nc.vector.tensor_add(gb[g][:], gb[g][:], bias_sb[g])

                def xh_mms(c):
                    c0 = c * HH
                    for ki, k in enumerate(KORD):
                        for hf in range(2):
                            vbase = ((k * 4 + 3) * 2 + hf) * FH
                            nc.tensor.matmul(
                                gx[c][hf * BLOC : (hf + 1) * BLOC, :],
                                hTs[k][:],
                                vsb[:, vbase + c0 : vbase + c0 + HH],
                                start=(ki == 0),
                                stop=(ki == KC - 1),
                                skip_group_check=True,
                            )
                    if has_bias:
                        nc.vector.tensor_add(
                            gx[c][:], gx[c][:], bias_sb[3][:, c0 : c0 + HH]
                        )

                bank_mms(0)  # rpre
                r = wpool.tile([P, FH], fp16, tag="r", name="r")
                nc.scalar.activation(r[:], gb[0][:], AF.Sigmoid)
                bank_mms(1)  # zpre
                zt = wpool.tile([P, FH], fp16, tag="z", name="zt")
                nc.scalar.activation(zt[:], gb[1][:], AF.Sigmoid)
                # u = z*h and w = 1-z run under the hh/xh matmul stream
                # (GPSIMD, off the DVE).  u is split into column halves so
                # h_new and the output DMA can also run split.
                w = wpool.tile([P, FH], fp16, tag="w", name="w")
                nc.gpsimd.tensor_scalar(w[:], zt[:], -1.0, 1.0, ALU.mult, ALU.add)
                uh = [
                    wpool.tile([P, HH], fp16, tag=f"u{c}", name=f"u{c}")
                    for c in range(2)
                ]
                for c in range(2):
                    nc.gpsimd.tensor_mul(
                        uh[c][:], zt[:, c * HH : (c + 1) * HH], hh_[c][:]
                    )
                bank_mms(2)  # hh
                # p split by column halves: p_A finishes ~130ns after the hh
                # bank and q_A chains onto it on the same engine
                ph = [
                    wpool.tile([P, HH], fp16, tag=f"p{c}", name=f"p{c}")
                    for c in range(2)
                ]
                for c in range(2):
                    nc.vector.tensor_mul(
                        ph[c][:], r[:, c * HH : (c + 1) * HH],
                        gb[2][:, c * HH : (c + 1) * HH],
                    )
                # xh bank split by column halves, half A first: q_A = p_A+xh_A
                # and the tanh_A -> m_A -> h_A -> transpose chain start ~0.5us
                # before the xh_B stream finishes.
                xh_mms(0)
                xh_mms(1)
                # q halves go into the retired r / z PSUM banks (separate
                # tiles, so tanh_A does not falsely wait on q_B; ScalarE
                # also reads PSUM faster than SBUF)
                qh = [gb[0], gb[1]]
                hha = [
                    wpool.tile([P, HH], fp16, tag=f"hh{c}", name=f"hha{c}")
                    for c in range(2)
                ]
                mh = [
                    wpool.tile([P, HH], fp16, tag=f"m{c}", name=f"m{c}")
                    for c in range(2)
                ]
                hnew = [spool.tile([P, HH], fp16, tag=f"h{c}", name=f"hn{c}") for c in range(2)]
                for c in range(2):
                    nc.vector.tensor_add(qh[c][:, :HH], ph[c][:], gx[c][:])
                for c in range(2):
                    cs = slice(c * HH, (c + 1) * HH)
                    nc.scalar.activation(hha[c][:], qh[c][:, :HH], AF.Tanh)
                    nc.vector.tensor_mul(mh[c][:], w[:, cs], hha[c][:])
                    nc.vector.tensor_add(hnew[c][:], mh[c][:], uh[c][:])

                if not last and NDUMMY:
                    # Dummy matmul pairs bridge the PE-idle window across the
                    # q->tanh->m->h_new tail.  Without them the HAM activity
                    # monitor parks the PE clock at K=4/8 (1.2 GHz) and every
                    # real matmul runs at half speed.  They re-read z-gate
                    # slices into a scratch PSUM tile that is never read.
                    dmy = ancpool.tile([P, FH], f32, tag="anc", name="dmy")
                    for i in range(NDUMMY):
                        k = KORD[i % KC]
                        for hf in range(2):
                            nc.tensor.matmul(
                                dmy[hf * BLOC : (hf + 1) * BLOC, :],
                                hTs[k][:],
                                vsb[
                                    :,
                                    ((k * 4 + 1) * 2 + hf) * FH : ((k * 4 + 1) * 2 + hf + 1) * FH,
                                ],
                                start=True,
                                stop=True,
                                skip_group_check=True,
                            )

                if not last:
                    # hT_new = h_new^T via 4 PE transposes + per-chunk CASTs
                    # into 4 separate stationary tiles (a single whole-tile
                    # fp16 PSUM copy spanning the 4 transpose groups faults
                    # the NEFF at runtime; per-chunk copies also give the
                    # next step's k-MMs finer-grained dependencies).  Column
                    # half A holds chunks {0,2}, half B {1,3} - KORD starts
                    # the next stream on chunks 0,2 while half B finishes.
                    # chunks {0,2} -> trpA tile, {1,3} -> trpB (2 PSUM banks)
                    trpt = [
                        trpool.tile([P, 2 * BLOC], fp16, tag=f"trp{n}", name=f"trp{n}")
                        for n in ("A", "B")
                    ]
                    hTs_new = [None] * KC
                    for k in KORD:
                        fold = k // 2
                        c = k % 2
                        tslice = trpt[c][:, fold * BLOC : (fold + 1) * BLOC]
                        nc.tensor.matmul(
                            tslice,
                            hnew[c][fold * BLOC : (fold + 1) * BLOC, :],
                            ident[fold * BLOC : (fold + 1) * BLOC, :],
                            is_transpose=True,
                            start=True,
                            stop=True,
                        )
                        hTk = spool.tile([P, BLOC], fp16, tag=f"hT{k}", name=f"hTn{k}")
                        nc.vector.tensor_copy(hTk[:], tslice)
                        hTs_new[k] = hTk
                    if DO_TRP:
                        hTs = hTs_new

                for c in range(2):
                    nc.sync.dma_start(out_d[:, t, c * HH : (c + 1) * HH], hnew[c][:])
                hh_ = hnew

    nc.compile()
    return nc


def kernel(x, W, U, b):
    from concourse.bass_utils import run_bass_kernel_spmd

    x = np.asarray(x, dtype=np.float32)
    W = np.asarray(W, dtype=np.float32)
    U = np.asarray(U, dtype=np.float32)
    b = np.asarray(b, dtype=np.float32)

    V, V0, bias = _prepare_weights(W, U, b)
    has_bias = bool(np.any(bias != 0.0))
    v_dev = _dev_layout(V).astype(_FP16)
    v0_dev = _dev_layout(V0).astype(_FP16)

    key = ("gru_v5_fp16", has_bias, T, NDUMMY, WARMUP, DO_TRP)
    if key not in _CACHE:
        _CACHE[key] = _build(has_bias, T)
    nc = _CACHE[key]

    in_maps = []
    for i in range(NCORES):
        xs = x[i * BLOC : (i + 1) * BLOC]  # [64, 512]
        xb = xs.astype(_FP16)
        xf = xb
        m = {
            "v0": v0_dev,
            "v": v_dev,
            # folded batch-major state: [p = fold*64+b, c] = x[b, fold*256+c]
            "h0": np.ascontiguousarray(
                xb.reshape(BLOC, 2, FH).transpose(1, 0, 2).reshape(P, FH)
            ),
            # transposed state: [p, k*64+b] = x[b, k*128+p]
            "h0T": np.ascontiguousarray(
                xf.reshape(BLOC, KC, P).transpose(2, 1, 0).reshape(P, KC * BLOC)
            ),
        }
        if has_bias:
            m["bias"] = _fold_bias(bias)
        in_maps.append(m)

    res = run_bass_kernel_spmd(
        nc, in_maps, core_ids=list(range(NCORES)), trace=TRACE, tmpdir=TMPDIR
    )
    LAST["exec_time_ns"] = res.exec_time_ns
    LAST["results"] = res
    outs = []
    for i in range(NCORES):
        o = res.results[i]["out"].astype(np.float32)  # [P, T, FH] fp16
        outs.append(
            o.reshape(2, BLOC, T, FH).transpose(1, 2, 0, 3).reshape(BLOC, T, D)
        )
    out = np.concatenate(outs, axis=0)
    return out.astype(np.float32)


# revision 23
# speedup vs baseline: 1.1232x; 1.1232x over previous
"""Autoregressive GRU on 8 TRN2 NeuronCores.

Data-parallel: batch B=512 is split as 64 rows per core; the small GRU
weights are replicated and the T=128 sequential loop runs locally per core.

Key algebra (Keras GRU, reset_after=True, gate order [z, r, h]):
  step 0:  inp = 0, h = x  ->  gx = b[0], gh = x @ U + b[1]
  step t>=1: inp == h      ->  gx + gh uses (W + U) for the z and r gates
so per step we need ONE matmul against a host-prefused weight matrix
  V  = [Wr+Ur | Wz+Uz | Uh | Wh]   (steps >= 1)   [D, 4D]
  V0 = [Ur   | Uz    | Uh | 0 ]   (step 0)       [D, 4D]
with per-gate PSUM banks in order [r, z, hh, xh], then
  r = sigmoid(rpre); z = sigmoid(zpre); hhat = tanh(xh + r*hh)
  h_new = (1-z)*hhat + z*h

Perf structure (v2 - col-tiled, fold-128 layout):
- Each M=64 matmul only fills half the 128-col PE array.  We issue the two
  256-wide halves of every gate row-block as a tile_position=(0,0)/(0,64)
  pair: the pair runs CONCURRENTLY on the two column halves of the array
  (4ns stagger), so a gate bank costs 4x~107ns instead of 4x~215ns.
- The pair's outputs land on PSUM partitions 0:64 and 64:128, i.e. every
  gate tensor is [128, 256] ("folded": partition = fold*64 + batch,
  col = feature % 256).  All elementwise work therefore runs at FD=256 on
  128 partitions - half the instruction time of the baseline's [64, 512].
- Bank order [r, z, hh, xh]: both sigmoids, u = z*h (GPSIMD) and w = 1-z
  run under the hh/xh matmul stream; the post-stream chain is only
  q = p+xh -> tanh -> m = w*hhat -> h_new = m+u -> 4 PE transposes ->
  one CAST to the fp16 stationary hT.
- Moving operands stay fp16 (exact weights, 1 cyc/row at N=256); the
  recurrent state is fp16 (~1e-2 rel overall).
- Warm-up identity transposes + two tiny regular matmuls mid-tail keep the
  PE HAM activity monitor from re-throttling the clock to 1.2 GHz.
"""

import numpy as np
import ml_dtypes

B, D, T = 512, 512, 128
NCORES = 8
BLOC = B // NCORES  # 64
P = 128
KC = D // P  # 4 K-chunks
FH = 256  # fold width (free dim of every folded [128, 256] tensor)
HH = FH // 2  # column half
GW = 4 * D  # 2048 gate columns: [r | z | hh | xh]

_FP16 = np.float16

# set by test harness to capture a profile; harmless when False
TRACE = False
TMPDIR = None
LAST = {}
# ablation flags (for debugging; all True in production)
NDUMMY = 6
WARMUP = True
DO_TRP = True


def _prepare_weights(W, U, b):
    """Host-side fusion. Gate order [r | z | hh | xh]."""
    Wz, Wr, Wh = W[:, :D], W[:, D : 2 * D], W[:, 2 * D :]
    Uz, Ur, Uh = U[:, :D], U[:, D : 2 * D], U[:, 2 * D :]
    V = np.concatenate([Wr + Ur, Wz + Uz, Uh, Wh], axis=1)  # [D, GW]
    V0 = np.concatenate([Ur, Uz, Uh, np.zeros_like(Wh)], axis=1)
    b0, b1 = b[0], b[1]
    bias = np.concatenate(
        [b0[D : 2 * D] + b1[D : 2 * D], b0[:D] + b1[:D], b1[2 * D :], b0[2 * D :]]
    )  # [GW], order [r | z | hh | xh]
    return V, V0, bias


def _dev_layout(V):
    # V_dev[p, ((k*4+g)*2+hf)*FH + c] = V[k*128+p, g*512 + hf*256 + c]
    return np.ascontiguousarray(
        V.reshape(KC, P, 4, 2, FH).transpose(1, 0, 2, 3, 4).reshape(P, KC * GW)
    )


def _fold_bias(bias):
    # folded per-gate [P, FH]: row p = fold*64+b (same for all b), col c
    out = np.zeros((4, P, FH), dtype=np.float32)
    for g in range(4):
        for hf in range(2):
            blk = bias[g * 512 + hf * 256 : g * 512 + (hf + 1) * 256]
            out[g, hf * BLOC : (hf + 1) * BLOC, :] = blk[None, :]
    return out


_CACHE = {}


def _build(has_bias: bool, T=T):
    import concourse.mybir as mybir
    import concourse.tile as tile
    from concourse import bacc
    from concourse.masks import make_identity

    f32 = mybir.dt.float32
    fp16 = mybir.dt.float16
    AF = mybir.ActivationFunctionType
    ALU = mybir.AluOpType

    nc = bacc.Bacc(
        "TRN2", target_bir_lowering=False, debug=False, num_devices=NCORES
    )
    v0_d = nc.dram_tensor("v0", [P, KC * GW], fp16, kind="ExternalInput").ap()
    v_d = nc.dram_tensor("v", [P, KC * GW], fp16, kind="ExternalInput").ap()
    h0_d = nc.dram_tensor("h0", [P, FH], fp16, kind="ExternalInput").ap()
    h0T_d = nc.dram_tensor("h0T", [P, KC * BLOC], fp16, kind="ExternalInput").ap()
    if has_bias:
        bias_d = nc.dram_tensor("bias", [4, P, FH], f32, kind="ExternalInput").ap()
    out_d = nc.dram_tensor("out", [P, T, FH], fp16, kind="ExternalOutput").ap()  # noqa: T param

    with tile.TileContext(nc) as tc:
        with (
            tc.tile_pool(name="const", bufs=1) as cpool,
            tc.tile_pool(name="state", bufs=2) as spool,
            tc.tile_pool(name="work", bufs=2) as wpool,
            tc.tile_pool(name="gates", bufs=1, space="PSUM") as gpool,
            tc.tile_pool(name="trp", bufs=1, space="PSUM") as trpool,
            tc.tile_pool(name="anc", bufs=1, space="PSUM") as ancpool,
        ):
            v0_sb = cpool.tile([P, KC * GW], fp16, tag="v0", name="v0_sb")
            v_sb = cpool.tile([P, KC * GW], fp16, tag="v", name="v_sb")
            ident = cpool.tile([P, BLOC], fp16, tag="ident", name="ident")
            nc.sync.dma_start(v0_sb[:], v0_d[:])
            make_identity(nc, ident[:BLOC, :])
            make_identity(nc, ident[BLOC:, :])

            hh_ = [spool.tile([P, HH], fp16, tag=f"h{c}", name=f"h{c}") for c in range(2)]
            hTs = [
                spool.tile([P, BLOC], fp16, tag=f"hT{k}", name=f"hT{k}")
                for k in range(KC)
            ]
            for c in range(2):
                nc.sync.dma_start(hh_[c][:], h0_d[:, c * HH : (c + 1) * HH])
            for k in range(KC):
                nc.sync.dma_start(hTs[k][:], h0T_d[:, k * BLOC : (k + 1) * BLOC])
            nc.sync.dma_start(v_sb[:], v_d[:])
            if has_bias:
                bias_sb = cpool.tile([4, P, FH], f32, tag="bias")
                nc.sync.dma_start(bias_sb[:], bias_d[:])

            # PE warm-up: dense transpose work that depends only on the
            # locally-built identity (not on any DMA) flips the HAM clock
            # gate to K=8/8 while the weight DMAs are still in flight.
            wu = trpool.tile([P, 2 * BLOC], fp16, tag="trpA", name="wu")
            for i in range(24 if WARMUP else 0):
                nc.tensor.matmul(
                    wu[:BLOC, (i % 2) * BLOC : (i % 2 + 1) * BLOC],
                    ident[:BLOC, :],
                    ident[:BLOC, :],
                    is_transpose=True,
                    start=True,
                    stop=True,
                )

            for t in range(T):
                vsb = v0_sb if t == 0 else v_sb
                last = t == T - 1
                # folded PSUM tiles: r / z banks [P, FH]; the xh bank is TWO
                # tiles (column halves) so q_A's dependency covers only the
                # 8 xh_A matmuls, not all 16 (Tile deps are tile-granular)
                gb = [
                    gpool.tile([P, FH], f32, tag=f"g{n}", name=f"g{n}")
                    for n in range(3)
                ]
                gx = [
                    gpool.tile([P, HH], f32, tag=f"gx{c}", name=f"gx{c}")
                    for c in range(2)
                ]

                KORD = (0, 2, 1, 3)  # chunk order matching the cast order

                def bank_mms(g):
                    for ki, k in enumerate(KORD):
                        for hf in range(2):
                            vbase = ((k * 4 + g) * 2 + hf) * FH
                            nc.tensor.matmul(
                                gb[g][hf * BLOC : (hf + 1) * BLOC, :],
                                hTs[k][:],
                                vsb[:, vbase : vbase + FH],
                                start=(ki == 0),
                                stop=(ki == KC - 1),
                                skip_group_check=True,
                            )
                    if has_bias:
                        nc.vector.tensor_add(gb[g][:], gb[g][:], bias_sb[g])

                def xh_mms(c):
                    c0 = c * HH
                    for ki, k in enumerate(KORD):
                        for hf in range(2):
                            vbase = ((k * 4 + 3) * 2 + hf) * FH
                            nc.tensor.matmul(
                                gx[c][hf * BLOC : (hf + 1) * BLOC, :],
                                hTs[k][:],
                                vsb[:, vbase + c0 : vbase + c0 + HH],
                                start=(ki == 0),
                                stop=(ki == KC - 1),
                                skip_group_check=True,
                            )
                    if has_bias:
                        nc.vector.tensor_add(
                            gx[c][:], gx[c][:], bias_sb[3][:, c0 : c0 + HH]
                        )

                bank_mms(0)  # rpre
                r = wpool.tile([P, FH], fp16, tag="r", name="r")
                nc.scalar.activation(r[:], gb[0][:], AF.Sigmoid)
                bank_mms(1)  # zpre
                zt = wpool.tile([P, FH], fp16, tag="z", name="zt")
                nc.scalar.activation(zt[:], gb[1][:], AF.Sigmoid)
                # u = z*h and w = 1-z run under the hh/xh matmul stream
                # (GPSIMD, off the DVE).  u is split into column halves so
                # h_new and the output DMA can also run split.
                w = wpool.tile([P, FH], fp16, tag="w", name="w")
                nc.gpsimd.tensor_scalar(w[:], zt[:], -1.0, 1.0, ALU.mult, ALU.add)
                uh = [
                    wpool.tile([P, HH], fp16, tag=f"u{c}", name=f"u{c}")
                    for c in range(2)
                ]
                for c in range(2):
                    nc.gpsimd.tensor_mul(
                        uh[c][:], zt[:, c * HH : (c + 1) * HH], hh_[c][:]
                    )
                bank_mms(2)  # hh
                # p split by column halves: p_A finishes ~130ns after the hh
                # bank and q_A chains onto it on the same engine
                ph = [
                    wpool.tile([P, HH], fp16, tag=f"p{c}", name=f"p{c}")
                    for c in range(2)
                ]
                for c in range(2):
                    nc.vector.tensor_mul(
                        ph[c][:], r[:, c * HH : (c + 1) * HH],
                        gb[2][:, c * HH : (c + 1) * HH],
                    )
                # xh bank split by column halves, half A first: q_A = p_A+xh_A
                # and the tanh_A -> m_A -> h_A -> transpose chain start ~0.5us
                # before the xh_B stream finishes.
                xh_mms(0)
                xh_mms(1)
                # q halves go into the retired r / z PSUM banks (separate
                # tiles, so tanh_A does not falsely wait on q_B; ScalarE
                # also reads PSUM faster than SBUF)
                qh = [gb[0], gb[1]]
                hha = [
                    wpool.tile([P, HH], fp16, tag=f"hh{c}", name=f"hha{c}")
                    for c in range(2)
                ]
                mh = [
                    wpool.tile([P, HH], fp16, tag=f"m{c}", name=f"m{c}")
                    for c in range(2)
                ]
                hnew = [spool.tile([P, HH], fp16, tag=f"h{c}", name=f"hn{c}") for c in range(2)]
                for c in range(2):
                    nc.vector.tensor_add(qh[c][:, :HH], ph[c][:], gx[c][:])
                for c in range(2):
                    cs = slice(c * HH, (c + 1) * HH)
                    nc.scalar.activation(hha[c][:], qh[c][:, :HH], AF.Tanh)
                    nc.vector.tensor_mul(mh[c][:], w[:, cs], hha[c][:])
                    nc.vector.tensor_add(hnew[c][:], mh[c][:], uh[c][:])

                if not last and NDUMMY:
                    # Dummy matmul pairs bridge the PE-idle window across the
                    # q->tanh->m->h_new tail.  Without them the HAM activity
                    # monitor parks the PE clock at K=4/8 (1.2 GHz) and every
                    # real matmul runs at half speed.  They re-read z-gate
                    # slices into a scratch PSUM tile that is never read.
                    dmy = ancpool.tile([P, FH], f32, tag="anc", name="dmy")
                    for i in range(NDUMMY):
                        k = KORD[i % KC]
                        for hf in range(2):
                            nc.tensor.matmul(
                                dmy[hf * BLOC : (hf + 1) * BLOC, :],
                                hTs[k][:],
                                vsb[
                                    :,
                                    ((k * 4 + 1) * 2 + hf) * FH : ((k * 4 + 1) * 2 + hf + 1) * FH,
                                ],
                                start=True,
                                stop=True,
                                skip_group_check=True,
                            )

                if not last:
                    # hT_new = h_new^T via 4 PE transposes + per-chunk CASTs
                    # into 4 separate stationary tiles (a single whole-tile
                    # fp16 PSUM copy spanning the 4 transpose groups faults
                    # the NEFF at runtime; per-chunk copies also give the
                    # next step's k-MMs finer-grained dependencies).  Column
                    # half A holds chunks {0,2}, half B {1,3} - KORD starts
                    # the next stream on chunks 0,2 while half B finishes.
                    # chunks {0,2} -> trpA tile, {1,3} -> trpB (2 PSUM banks)
                    trpt = [
                        trpool.tile([P, 2 * BLOC], fp16, tag=f"trp{n}", name=f"trp{n}")
                        for n in ("A", "B")
                    ]
                    hTs_new = [None] * KC
                    for k in KORD:
                        fold = k // 2
                        c = k % 2
                        tslice = trpt[c][:, fold * BLOC : (fold + 1) * BLOC]
                        nc.tensor.matmul(
                            tslice,
                            hnew[c][fold * BLOC : (fold + 1) * BLOC, :],
                            ident[fold * BLOC : (fold + 1) * BLOC, :],
                            is_transpose=True,
                            start=True,
                            stop=True,
                        )
                        hTk = spool.tile([P, BLOC], fp16, tag=f"hT{k}", name=f"hTn{k}")
                        nc.vector.tensor_copy(hTk[:], tslice)
                        hTs_new[k] = hTk
                    if DO_TRP:
                        hTs = hTs_new

                for c in range(2):
                    nc.sync.dma_start(out_d[:, t, c * HH : (c + 1) * HH], hnew[c][:])
                hh_ = hnew

    nc.compile()
    return nc


def kernel(x, W, U, b):
    from concourse.bass_utils import run_bass_kernel_spmd

    x = np.asarray(x, dtype=np.float32)
    W = np.asarray(W, dtype=np.float32)
    U = np.asarray(U, dtype=np.float32)
    b = np.asarray(b, dtype=np.float32)

    V, V0, bias = _prepare_weights(W, U, b)
    has_bias = bool(np.any(bias != 0.0))
    v_dev = _dev_layout(V).astype(_FP16)
    v0_dev = _dev_layout(V0).astype(_FP16)

    key = ("gru_v5_fp16", has_bias, T, NDUMMY, WARMUP, DO_TRP)
    if key not in _CACHE:
        _CACHE[key] = _build(has_bias, T)
    nc = _CACHE[key]

    in_maps = []
    for i in range(NCORES):
        xs = x[i * BLOC : (i + 1) * BLOC]  # [64, 512]
        xb = xs.astype(_FP16)
        xf = xb
        m = {
            "v0": v0_dev,
            "v": v_dev,
            # folded batch-major state: [p = fold*64+b, c] = x[b, fold*256+c]
            "h0": np.ascontiguousarray(
                xb.reshape(BLOC, 2, FH).transpose(1, 0, 2).reshape(P, FH)
            ),
            # transposed state: [p, k*64+b] = x[b, k*128+p]
            "h0T": np.ascontiguousarray(
                xf.reshape(BLOC, KC, P).transpose(2, 1, 0).reshape(P, KC * BLOC)
            ),
        }
        if has_bias:
            m["bias"] = _fold_bias(bias)
        in_maps.append(m)

    res = run_bass_kernel_spmd(
        nc, in_maps, core_ids=list(range(NCORES)), trace=TRACE, tmpdir=TMPDIR
    )
    LAST["exec_time_ns"] = res.exec_time_ns
    LAST["results"] = res
    outs = []
    for i in range(NCORES):
        o = res.results[i]["out"].astype(np.float32)  # [P, T, FH] fp16
        outs.append(
            o.reshape(2, BLOC, T, FH).transpose(1, 2, 0, 3).reshape(BLOC, T, D)
        )
    out = np.concatenate(outs, axis=0)
    return out.astype(np.float32)


# revision 24
# speedup vs baseline: 1.1447x; 1.0192x over previous
"""Autoregressive GRU on 8 TRN2 NeuronCores.

Data-parallel: batch B=512 is split as 64 rows per core; the small GRU
weights are replicated and the T=128 sequential loop runs locally per core.

Key algebra (Keras GRU, reset_after=True, gate order [z, r, h]):
  step 0:  inp = 0, h = x  ->  gx = b[0], gh = x @ U + b[1]
  step t>=1: inp == h      ->  gx + gh uses (W + U) for the z and r gates
so per step we need ONE matmul against a host-prefused weight matrix
  V  = [Wr+Ur | Wz+Uz | Uh | Wh]   (steps >= 1)   [D, 4D]
  V0 = [Ur   | Uz    | Uh | 0 ]   (step 0)       [D, 4D]
with per-gate PSUM banks in order [r, z, hh, xh], then
  r = sigmoid(rpre); z = sigmoid(zpre); hhat = tanh(xh + r*hh)
  h_new = (1-z)*hhat + z*h

Perf structure (~660 us, 5.2 us/step, vs 1078 us baseline; rel err 2e-3):
- Everything fp16 (weights, state, gate intermediates): fp16's 10 mantissa
  bits cut the per-step state-rounding error ~8x vs bf16, which is what
  allows quantizing the weights at all (bf16 weights alone measure 1.9e-2,
  right at the tolerance).  f32r matmuls reject tile_position, fp16 passes.
- Col-tiled matmul pairs: each M=64 matmul fills half the 128-col PE
  array, so the two 256-col halves of every gate row-block are issued as a
  tile_position (0,0)/(0,64) pair that runs CONCURRENTLY on the two array
  halves (measured 3-6ns stagger, ~107ns/pair warm).  Outputs land on PSUM
  partitions 0:64 / 64:128: every gate tensor is [128, 256] "folded"
  (partition = fold*64 + batch), halving elementwise FD vs [64, 512].
- Bank order [r, z, hh, xh]: both sigmoids, w = 1-z and u = z*h (GPSIMD)
  hide under the hh/xh stream.  The xh bank is 2 PSUM tiles (column
  halves, half A streamed first at N=128) and q/tanh/m/h_new are split
  into A/B column halves, so the A-chain q_A -> tanh_A -> m_A -> h_A ->
  transpose(chunks 0,2) -> cast starts ~0.5 us before the stream ends and
  the next stream (k-order 0,2,1,3) starts on chunk 0's cast while the
  B-chain finishes chunks 1,3.
- Tile dependency tracking is TILE-granular: every half/chunk gets its own
  tile (q halves reuse the retired r/z PSUM banks; transposes use two PSUM
  tiles; stationaries are 4 separate hT tiles) - sharing one tile creates
  false WARs that stall the PE stream by 300-500ns each.
- HAM clock gate: any per-step PE idle window re-throttles the PE to
  1.2 GHz and the whole 3.4us-cold-window stream then runs at half speed.
  6 dummy matmul pairs (never-read scratch PSUM) bridge the tail window;
  more is WORSE (12 pairs measured +80us - P0 power-state downclock).
- A single whole-tile fp16 PSUM copy spanning 4 transpose groups faults
  the NEFF at runtime - the hT casts must be per-chunk copies.
- Output is DMA'd directly from the fp16 h_new halves ([P, T, 256] fp16
  dram, 1 partition = fold*64+batch); the f32 cast + unfold happen on host.
"""

import numpy as np

B, D, T = 512, 512, 128
NCORES = 8
BLOC = B // NCORES  # 64
P = 128
KC = D // P  # 4 K-chunks
FH = 256  # fold width (free dim of every folded [128, 256] tensor)
HH = FH // 2  # column half
GW = 4 * D  # 2048 gate columns: [r | z | hh | xh]

_FP16 = np.float16

# set by test harness to capture a profile; harmless when False
TRACE = False
TMPDIR = None
LAST = {}
# ablation flags (for debugging; all True in production)
NDUMMY = 6
WARMUP = True
DO_TRP = True


def _prepare_weights(W, U, b):
    """Host-side fusion. Gate order [r | z | hh | xh]."""
    Wz, Wr, Wh = W[:, :D], W[:, D : 2 * D], W[:, 2 * D :]
    Uz, Ur, Uh = U[:, :D], U[:, D : 2 * D], U[:, 2 * D :]
    V = np.concatenate([Wr + Ur, Wz + Uz, Uh, Wh], axis=1)  # [D, GW]
    V0 = np.concatenate([Ur, Uz, Uh, np.zeros_like(Wh)], axis=1)
    b0, b1 = b[0], b[1]
    bias = np.concatenate(
        [b0[D : 2 * D] + b1[D : 2 * D], b0[:D] + b1[:D], b1[2 * D :], b0[2 * D :]]
    )  # [GW], order [r | z | hh | xh]
    return V, V0, bias


def _dev_layout(V):
    # V_dev[p, ((k*4+g)*2+hf)*FH + c] = V[k*128+p, g*512 + hf*256 + c]
    return np.ascontiguousarray(
        V.reshape(KC, P, 4, 2, FH).transpose(1, 0, 2, 3, 4).reshape(P, KC * GW)
    )


def _fold_bias(bias):
    # folded per-gate [P, FH]: row p = fold*64+b (same for all b), col c
    out = np.zeros((4, P, FH), dtype=np.float32)
    for g in range(4):
        for hf in range(2):
            blk = bias[g * 512 + hf * 256 : g * 512 + (hf + 1) * 256]
            out[g, hf * BLOC : (hf + 1) * BLOC, :] = blk[None, :]
    return out


_CACHE = {}


def _build(has_bias: bool, T=T):
    import concourse.mybir as mybir
    import concourse.tile as tile
    from concourse import bacc
    from concourse.masks import make_identity

    f32 = mybir.dt.float32
    fp16 = mybir.dt.float16
    AF = mybir.ActivationFunctionType
    ALU = mybir.AluOpType

    nc = bacc.Bacc(
        "TRN2", target_bir_lowering=False, debug=False, num_devices=NCORES
    )
    v0_d = nc.dram_tensor("v0", [P, KC * GW], fp16, kind="ExternalInput").ap()
    v_d = nc.dram_tensor("v", [P, KC * GW], fp16, kind="ExternalInput").ap()
    h0_d = nc.dram_tensor("h0", [P, FH], fp16, kind="ExternalInput").ap()
    h0T_d = nc.dram_tensor("h0T", [P, KC * BLOC], fp16, kind="ExternalInput").ap()
    if has_bias:
        bias_d = nc.dram_tensor("bias", [4, P, FH], f32, kind="ExternalInput").ap()
    out_d = nc.dram_tensor("out", [P, T, FH], fp16, kind="ExternalOutput").ap()  # noqa: T param

    with tile.TileContext(nc) as tc:
        with (
            tc.tile_pool(name="const", bufs=1) as cpool,
            tc.tile_pool(name="state", bufs=2) as spool,
            tc.tile_pool(name="work", bufs=2) as wpool,
            tc.tile_pool(name="gates", bufs=1, space="PSUM") as gpool,
            tc.tile_pool(name="trp", bufs=1, space="PSUM") as trpool,
            tc.tile_pool(name="anc", bufs=1, space="PSUM") as ancpool,
        ):
            v0_sb = cpool.tile([P, KC * GW], fp16, tag="v0", name="v0_sb")
            v_sb = cpool.tile([P, KC * GW], fp16, tag="v", name="v_sb")
            ident = cpool.tile([P, BLOC], fp16, tag="ident", name="ident")
            nc.sync.dma_start(v0_sb[:], v0_d[:])
            make_identity(nc, ident[:BLOC, :])
            make_identity(nc, ident[BLOC:, :])

            hh_ = [spool.tile([P, HH], fp16, tag=f"h{c}", name=f"h{c}") for c in range(2)]
            hTs = [
                spool.tile([P, BLOC], fp16, tag=f"hT{k}", name=f"hT{k}")
                for k in range(KC)
            ]
            for c in range(2):
                nc.sync.dma_start(hh_[c][:], h0_d[:, c * HH : (c + 1) * HH])
            for k in range(KC):
                nc.sync.dma_start(hTs[k][:], h0T_d[:, k * BLOC : (k + 1) * BLOC])
            nc.sync.dma_start(v_sb[:], v_d[:])
            if has_bias:
                bias_sb = cpool.tile([4, P, FH], f32, tag="bias")
                nc.sync.dma_start(bias_sb[:], bias_d[:])

            # PE warm-up: dense transpose work that depends only on the
            # locally-built identity (not on any DMA) flips the HAM clock
            # gate to K=8/8 while the weight DMAs are still in flight.
            wu = trpool.tile([P, 2 * BLOC], fp16, tag="trpA", name="wu")
            for i in range(24 if WARMUP else 0):
                nc.tensor.matmul(
                    wu[:BLOC, (i % 2) * BLOC : (i % 2 + 1) * BLOC],
                    ident[:BLOC, :],
                    ident[:BLOC, :],
                    is_transpose=True,
                    start=True,
                    stop=True,
                )

            for t in range(T):
                vsb = v0_sb if t == 0 else v_sb
                last = t == T - 1
                # folded PSUM tiles: r / z banks [P, FH]; the xh bank is TWO
                # tiles (column halves) so q_A's dependency covers only the
                # 8 xh_A matmuls, not all 16 (Tile deps are tile-granular)
                gb = [
                    gpool.tile([P, FH], f32, tag=f"g{n}", name=f"g{n}")
                    for n in range(3)
                ]
                gx = [
                    gpool.tile([P, HH], f32, tag=f"gx{c}", name=f"gx{c}")
                    for c in range(2)
                ]

                KORD = (0, 2, 1, 3)  # chunk order matching the cast order

                def bank_mms(g):
                    for ki, k in enumerate(KORD):
                        for hf in range(2):
                            vbase = ((k * 4 + g) * 2 + hf) * FH
                            nc.tensor.matmul(
                                gb[g][hf * BLOC : (hf + 1) * BLOC, :],
                                hTs[k][:],
                                vsb[:, vbase : vbase + FH],
                                start=(ki == 0),
                                stop=(ki == KC - 1),
                                skip_group_check=True,
                            )
                    if has_bias:
                        nc.vector.tensor_add(gb[g][:], gb[g][:], bias_sb[g])

                def xh_mms(c):
                    c0 = c * HH
                    for ki, k in enumerate(KORD):
                        for hf in range(2):
                            vbase = ((k * 4 + 3) * 2 + hf) * FH
                            nc.tensor.matmul(
                                gx[c][hf * BLOC : (hf + 1) * BLOC, :],
                                hTs[k][:],
                                vsb[:, vbase + c0 : vbase + c0 + HH],
                                start=(ki == 0),
                                stop=(ki == KC - 1),
                                skip_group_check=True,
                            )
                    if has_bias:
                        nc.vector.tensor_add(
                            gx[c][:], gx[c][:], bias_sb[3][:, c0 : c0 + HH]
                        )

                bank_mms(0)  # rpre
                r = wpool.tile([P, FH], fp16, tag="r", name="r")
                nc.scalar.activation(r[:], gb[0][:], AF.Sigmoid)
                bank_mms(1)  # zpre
                zt = wpool.tile([P, FH], fp16, tag="z", name="zt")
                nc.scalar.activation(zt[:], gb[1][:], AF.Sigmoid)
                # u = z*h and w = 1-z run under the hh/xh matmul stream
                # (GPSIMD, off the DVE).  u is split into column halves so
                # h_new and the output DMA can also run split.
                w = wpool.tile([P, FH], fp16, tag="w", name="w")
                nc.gpsimd.tensor_scalar(w[:], zt[:], -1.0, 1.0, ALU.mult, ALU.add)
                uh = [
                    wpool.tile([P, HH], fp16, tag=f"u{c}", name=f"u{c}")
                    for c in range(2)
                ]
                for c in range(2):
                    nc.gpsimd.tensor_mul(
                        uh[c][:], zt[:, c * HH : (c + 1) * HH], hh_[c][:]
                    )
                bank_mms(2)  # hh
                # p split by column halves: p_A finishes ~130ns after the hh
                # bank and q_A chains onto it on the same engine
                ph = [
                    wpool.tile([P, HH], fp16, tag=f"p{c}", name=f"p{c}")
                    for c in range(2)
                ]
                for c in range(2):
                    nc.vector.tensor_mul(
                        ph[c][:], r[:, c * HH : (c + 1) * HH],
                        gb[2][:, c * HH : (c + 1) * HH],
                    )
                # xh bank split by column halves, half A first: q_A = p_A+xh_A
                # and the tanh_A -> m_A -> h_A -> transpose chain start ~0.5us
                # before the xh_B stream finishes.
                xh_mms(0)
                xh_mms(1)
                # q halves go into the retired r / z PSUM banks (separate
                # tiles, so tanh_A does not falsely wait on q_B; ScalarE
                # also reads PSUM faster than SBUF)
                qh = [gb[0], gb[1]]
                hha = [
                    wpool.tile([P, HH], fp16, tag=f"hh{c}", name=f"hha{c}")
                    for c in range(2)
                ]
                mh = [
                    wpool.tile([P, HH], fp16, tag=f"m{c}", name=f"m{c}")
                    for c in range(2)
                ]
                hnew = [spool.tile([P, HH], fp16, tag=f"h{c}", name=f"hn{c}") for c in range(2)]
                for c in range(2):
                    nc.vector.tensor_add(qh[c][:, :HH], ph[c][:], gx[c][:])
                for c in range(2):
                    cs = slice(c * HH, (c + 1) * HH)
                    nc.scalar.activation(hha[c][:], qh[c][:, :HH], AF.Tanh)
                    nc.vector.tensor_mul(mh[c][:], w[:, cs], hha[c][:])
                    nc.vector.tensor_add(hnew[c][:], mh[c][:], uh[c][:])

                if not last and NDUMMY:
                    # Dummy matmul pairs bridge the PE-idle window across the
                    # q->tanh->m->h_new tail.  Without them the HAM activity
                    # monitor parks the PE clock at K=4/8 (1.2 GHz) and every
                    # real matmul runs at half speed.  They re-read z-gate
                    # slices into a scratch PSUM tile that is never read.
                    dmy = ancpool.tile([P, FH], f32, tag="anc", name="dmy")
                    for i in range(NDUMMY):
                        k = KORD[i % KC]
                        for hf in range(2):
                            nc.tensor.matmul(
                                dmy[hf * BLOC : (hf + 1) * BLOC, :],
                                hTs[k][:],
                                vsb[
                                    :,
                                    ((k * 4 + 1) * 2 + hf) * FH : ((k * 4 + 1) * 2 + hf + 1) * FH,
                                ],
                                start=True,
                                stop=True,
                                skip_group_check=True,
                            )

                if not last:
                    # hT_new = h_new^T via 4 PE transposes + per-chunk CASTs
                    # into 4 separate stationary tiles (a single whole-tile
                    # fp16 PSUM copy spanning the 4 transpose groups faults
                    # the NEFF at runtime; per-chunk copies also give the
                    # next step's k-MMs finer-grained dependencies).  Column
                    # half A holds chunks {0,2}, half B {1,3} - KORD starts
                    # the next stream on chunks 0,2 while half B finishes.
                    # chunks {0,2} -> trpA tile, {1,3} -> trpB (2 PSUM banks)
                    trpt = [
                        trpool.tile([P, 2 * BLOC], fp16, tag=f"trp{n}", name=f"trp{n}")
                        for n in ("A", "B")
                    ]
                    hTs_new = [None] * KC
                    for k in KORD:
                        fold = k // 2
                        c = k % 2
                        tslice = trpt[c][:, fold * BLOC : (fold + 1) * BLOC]
                        nc.tensor.matmul(
                            tslice,
                            hnew[c][fold * BLOC : (fold + 1) * BLOC, :],
                            ident[fold * BLOC : (fold + 1) * BLOC, :],
                            is_transpose=True,
                            start=True,
                            stop=True,
                        )
                        hTk = spool.tile([P, BLOC], fp16, tag=f"hT{k}", name=f"hTn{k}")
                        nc.vector.tensor_copy(hTk[:], tslice)
                        hTs_new[k] = hTk
                    if DO_TRP:
                        hTs = hTs_new

                for c in range(2):
                    nc.sync.dma_start(out_d[:, t, c * HH : (c + 1) * HH], hnew[c][:])
                hh_ = hnew

    nc.compile()
    return nc


def kernel(x, W, U, b):
    from concourse.bass_utils import run_bass_kernel_spmd

    x = np.asarray(x, dtype=np.float32)
    W = np.asarray(W, dtype=np.float32)
    U = np.asarray(U, dtype=np.float32)
    b = np.asarray(b, dtype=np.float32)

    V, V0, bias = _prepare_weights(W, U, b)
    has_bias = bool(np.any(bias != 0.0))
    v_dev = _dev_layout(V).astype(_FP16)
    v0_dev = _dev_layout(V0).astype(_FP16)

    key = ("gru_v5_fp16", has_bias, T, NDUMMY, WARMUP, DO_TRP)
    if key not in _CACHE:
        _CACHE[key] = _build(has_bias, T)
    nc = _CACHE[key]

    in_maps = []
    for i in range(NCORES):
        xs = x[i * BLOC : (i + 1) * BLOC]  # [64, 512]
        xb = xs.astype(_FP16)
        xf = xb
        m = {
            "v0": v0_dev,
            "v": v_dev,
            # folded batch-major state: [p = fold*64+b, c] = x[b, fold*256+c]
            "h0": np.ascontiguousarray(
                xb.reshape(BLOC, 2, FH).transpose(1, 0, 2).reshape(P, FH)
            ),
            # transposed state: [p, k*64+b] = x[b, k*128+p]
            "h0T": np.ascontiguousarray(
                xf.reshape(BLOC, KC, P).transpose(2, 1, 0).reshape(P, KC * BLOC)
            ),
        }
        if has_bias:
            m["bias"] = _fold_bias(bias)
        in_maps.append(m)

    res = run_bass_kernel_spmd(
        nc, in_maps, core_ids=list(range(NCORES)), trace=TRACE, tmpdir=TMPDIR
    )
    LAST["exec_time_ns"] = res.exec_time_ns
    LAST["results"] = res
    outs = []
    for i in range(NCORES):
        o = res.results[i]["out"].astype(np.float32)  # [P, T, FH] fp16
        outs.append(
            o.reshape(2, BLOC, T, FH).transpose(1, 2, 0, 3).reshape(BLOC, T, D)
        )
    out = np.concatenate(outs, axis=0)
    return out.astype(np.float32)
